# revision 8
# baseline (speedup 1.0000x reference)
"""Distributed Trainium2 Bass kernel for nn_AtomEncoder (NNConv/GRU message passing).

Strategy (8 cores, SPMD):
  - Edges are dst-sharded: core c owns all edges whose dst is in [2500c, 2500(c+1)),
    sorted by dst, grouped into 20 windows of 128 nodes, padded to a fixed
    tiles-per-window so every core runs an identical instruction stream.
  - Node state h lives as a full replicated HBM table [8*2560, 64] (row padded to
    256B for dma_gather); each layer's updated shard is AllGathered.
  - Per edge tile (128 edges):
      ew  = r_aug @ W_e2aug            (PE, bf16; r = relu(edge-MLP), recomputed-free)
      z2  = ew * h[src] (broadcast)    (DVE)
      ZA += S^T @ z2                   (PE; S = one-hot(dst_local) built by is_equal)
    The i-reduction (msg = sum_i h_i * ew[i,o]) is deferred past the scatter:
    once per 128-node window, agg[v,o] = sum_i ZA[v,(o,i)]  (DVE strided reduce).
  - GRU runs feature-major (h^T [32, 2560]) so biases are per-partition ACT ops.
  - Graph mean via one-hot matmul accumulated in PSUM; per-core partials are
    summed and divided on the host (unshard step).
"""

import os
import sys

import numpy as np
import ml_dtypes

for _p in ("/opt/trn_rl_repo", "/root/.axon_site/_ro/trn_rl_repo"):
    if os.path.isdir(_p) and _p not in sys.path:
        sys.path.insert(0, _p)

import concourse.bacc as bacc  # noqa: E402
import concourse.bass as bass  # noqa: E402
import concourse.mybir as mybir  # noqa: E402
import concourse.tile as tile  # noqa: E402
from concourse.bass_utils import run_bass_kernel_spmd  # noqa: E402

NCORES = 8
N, E, B, H = 20000, 320000, 128, 32
NPC = N // NCORES          # 2500 nodes per core
WIN = 20                   # 128-node windows per core
NPAD = WIN * 128           # 2560 padded nodes per core
NTAB = NCORES * NPAD       # 20480 rows in the replicated h table

F32 = mybir.dt.float32
BF16 = mybir.dt.bfloat16
I16 = mybir.dt.int16

_graph_cache = {}


def _build_graph(t_w, trace=False):
    """Build the SPMD Bass graph. t_w = tiles per 128-node window (even)."""
    dbg = int(os.environ.get("KDBG", "0") or 0)
    nt = WIN * t_w               # edge tiles per core
    e_pad = nt * 128             # padded edges per core
    nb = e_pad // 1024           # 8-tile transpose batches for e_feat
    ch_tiles = 4                 # gather chunk = 4 tiles (512 idxs; 2304 crashes SWDGE)
    n_ch = nt // 4               # gather chunks per layer
    g = ch_tiles * 128           # idxs per gather chunk

    nc = bacc.Bacc("TRN2", target_bir_lowering=False, debug=False,
                   num_devices=NCORES)

    def inp(name, shape, dt=F32):
        return nc.dram_tensor(name, list(shape), dt, kind="ExternalInput").ap()

    nfeat_d = inp("nfeat", (NPAD, 64))
    efeat_d = inp("efeat", (e_pad, 16))
    srcidx_d = inp("srcidx", (128, e_pad // 16), I16)
    dstl_d = inp("dstl", (128, nt))
    invd_d = inp("invd", (128, WIN))
    gid_d = inp("gid", (128, WIN))
    iota_d = inp("iota", (128, 128))
    eye_d = inp("eye", (128, 128))
    wa_d = inp("wa", (65, 32))
    wb_d = inp("wb", (16, 32))
    bb_d = inp("bb", (32, 1))
    we1_d = inp("we1", (33, 32))
    we2_d = inp("we2", (33, 1024), BF16)
    wih_d = inp("wih", (33, 96))
    whh_d = inp("whh", (33, 96))
    out_d = nc.dram_tensor("out", [128, 32], F32, kind="ExternalOutput").ap()

    AF = mybir.ActivationFunctionType
    ALU = mybir.AluOpType
    RG = [list(range(NCORES))]

    with tile.TileContext(nc) as tc:
        with tc.tile_pool(name="constp", bufs=1) as constp, \
             tc.tile_pool(name="bigp", bufs=1) as bigp, \
             tc.tile_pool(name="sbp", bufs=2) as sbp, \
             tc.tile_pool(name="sbp3", bufs=3) as sbp3, \
             tc.tile_pool(name="grup", bufs=1) as grup, \
             tc.tile_pool(name="dramp", bufs=1, space="DRAM") as dramp, \
             tc.tile_pool(name="psA", bufs=2, space="PSUM") as psA, \
             tc.tile_pool(name="psB", bufs=1, space="PSUM") as psB, \
             tc.tile_pool(name="psC", bufs=2, space="PSUM") as psC:

            # ---- constants to SBUF ----
            def load_const(name, ap, shape, dt=F32):
                t = constp.tile(list(shape), dt, name=name)
                nc.sync.dma_start(t[:], ap)
                return t

            iota_sb = load_const("iota_sb", iota_d, (128, 128))
            eye_sb = load_const("eye_sb", eye_d, (128, 128))
            wa_sb = load_const("wa_sb", wa_d, (65, 32))
            wb_sb = load_const("wb_sb", wb_d, (16, 32))
            bb_sb = load_const("bb_sb", bb_d, (32, 1))
            we1_sb = load_const("we1_sb", we1_d, (33, 32))
            we2_sb = load_const("we2_sb", we2_d, (33, 1024), BF16)
            wih_sb = load_const("wih_sb", wih_d, (33, 96))
            whh_sb = load_const("whh_sb", whh_d, (33, 96))
            invd_sb = load_const("invd_sb", invd_d, (128, WIN))
            gid_sb = load_const("gid_sb", gid_d, (128, WIN))
            dstl_sb = load_const("dstl_sb", dstl_d, (128, nt))
            srcidx_sb = load_const("srcidx_sb", srcidx_d, (128, e_pad // 16), I16)

            # ---- persistent buffers ----
            r_augT = bigp.tile([33, e_pad], BF16, name="r_augT")
            hT_a = bigp.tile([33, NPAD], F32, name="hT_a")
            hT_b = bigp.tile([33, NPAD], F32, name="hT_b")
            mT_aug = bigp.tile([33, NPAD], F32, name="mT_aug")
            nc.vector.memset(r_augT[32:33, :], 1.0)
            nc.vector.memset(hT_a[32:33, :], 1.0)
            nc.vector.memset(hT_b[32:33, :], 1.0)
            nc.vector.memset(mT_aug[32:33, :], 1.0)

            htab_sh0 = dramp.tile([NPAD, 64], F32, name="htab_sh0")
            htab_sh1 = dramp.tile([NPAD, 64], F32, name="htab_sh1")
            htab0 = dramp.tile([NTAB, 64], F32, name="htab0", addr_space="Shared")
            htab1 = dramp.tile([NTAB, 64], F32, name="htab1", addr_space="Shared")

            # ---- P1: initial node embedding h0 ----
            for w in range(WIN):
                nf_t = sbp.tile([128, 64], F32, tag="nf_t")
                nc.sync.dma_start(nf_t[:], nfeat_d[w * 128:(w + 1) * 128, :])
                tr_ps = psC.tile([64, 128], F32, tag="misc")
                nc.tensor.transpose(tr_ps[:], nf_t[:], eye_sb[:, :])
                nfT = sbp.tile([65, 128], F32, tag="nfT")
                nc.scalar.copy(nfT[0:64, :], tr_ps[:])
                nc.vector.memset(nfT[64:65, :], 1.0)
                # feature-major h0^T into hT_a
                h0T_ps = psC.tile([32, 128], F32, tag="misc")
                nc.tensor.matmul(h0T_ps[:], wa_sb[:], nfT[:])
                nc.scalar.copy(hT_a[0:32, w * 128:(w + 1) * 128], h0T_ps[:])
                # node-major h0 into the gather table shard
                h0_ps = psC.tile([128, 32], F32, tag="misc")
                nc.tensor.matmul(h0_ps[:], nfT[:], wa_sb[:])
                h0_sb = sbp.tile([128, 32], F32, tag="hn_sb")
                nc.scalar.copy(h0_sb[:], h0_ps[:])
                nc.sync.dma_start(htab_sh0[w * 128:(w + 1) * 128, 0:32], h0_sb[:])

            nc.gpsimd.collective_compute(
                "AllGather", ALU.bypass, replica_groups=RG,
                ins=[htab_sh0.opt()], outs=[htab0.opt()])

            # ---- P2: edge MLP r = relu(h_bond @ W_e1 + b_e1), feature-major ----
            for t in range(nt if (dbg == 0 or dbg >= 2) else 0):
                col = t * 128
                ef_t = sbp3.tile([128, 16], F32, tag="ef_t")
                nc.sync.dma_start(ef_t[:], efeat_d[col:col + 128, :])
                tr2_ps = psC.tile([16, 128], F32, tag="misc")
                nc.tensor.transpose(tr2_ps[:], ef_t[:], eye_sb[:, :])
                efT = sbp3.tile([16, 128], F32, tag="efT")
                nc.scalar.copy(efT[:], tr2_ps[:])
                hb_ps = psC.tile([32, 128], F32, tag="misc")
                nc.tensor.matmul(hb_ps[:], wb_sb[:], efT[:])
                hba = sbp3.tile([33, 128], F32, tag="hba")
                nc.vector.tensor_scalar_add(hba[0:32, :], hb_ps[:], bb_sb[:])
                nc.vector.memset(hba[32:33, :], 1.0)
                r_ps = psC.tile([32, 128], F32, tag="misc")
                nc.tensor.matmul(r_ps[:], we1_sb[:], hba[:])
                nc.scalar.activation(r_augT[0:32, col:col + 128], r_ps[:], AF.Relu)

            # ---- P3: message-passing layers ----
            hT_cur, hT_new = hT_a, hT_b
            htabs = [htab0, htab1]
            htab_shs = [htab_sh0, htab_sh1]
            n_layers = 2 if (dbg == 0 or dbg >= 5) else (1 if dbg >= 3 else 0)
            for layer in range(n_layers):
                htab = htabs[layer]
                nc.vector.memset(mT_aug[32:33, :], 1.0)
                hs_chunks = []
                for ch in range(n_ch):
                    hs = sbp.tile([128, ch_tiles, 64], F32, tag="hs")
                    nc.gpsimd.dma_gather(
                        hs[:], htab[:],
                        srcidx_sb[:, ch * (g // 16):(ch + 1) * (g // 16)],
                        g, g, 64)
                    hs_chunks.append(hs)

                for w in range(WIN if (dbg == 0 or dbg >= 4) else 0):
                    za_ps = psB.tile([128, 1024], F32, tag="za")
                    for i in range(t_w):
                        t = w * t_w + i
                        hs = hs_chunks[t // ch_tiles]
                        tt = t % ch_tiles
                        ew_ps = psA.tile([128, 1024], F32, tag="ew")
                        lhs_r = r_augT[0:33, t * 128:(t + 1) * 128]
                        nc.tensor.matmul(ew_ps[:, 0:512], lhs_r, we2_sb[:, 0:512])
                        nc.tensor.matmul(ew_ps[:, 512:1024], lhs_r, we2_sb[:, 512:1024])
                        z2 = sbp3.tile([128, 1024], BF16, tag="z2")
                        hb = hs[:, tt, 0:32].unsqueeze(1).broadcast_to([128, 32, 32])
                        nc.vector.tensor_tensor(
                            z2[:].rearrange("p (o i) -> p o i", o=32, i=32),
                            ew_ps[:].rearrange("p (o i) -> p o i", o=32, i=32),
                            hb, ALU.mult)
                        s_oh = sbp3.tile([128, 128], BF16, tag="s_oh")
                        nc.vector.tensor_scalar(
                            s_oh[:], iota_sb[:], dstl_sb[:, t:t + 1], None,
                            ALU.is_equal)
                        nc.tensor.matmul(za_ps[:, 0:512], s_oh[:], z2[:, 0:512],
                                         start=(i == 0), stop=(i == t_w - 1))
                        nc.tensor.matmul(za_ps[:, 512:1024], s_oh[:], z2[:, 512:1024],
                                         start=(i == 0), stop=(i == t_w - 1))
                    # window epilogue: reduce over i, scale by 1/deg, relu, transpose
                    agg = sbp.tile([128, 32], F32, tag="agg")
                    nc.vector.tensor_reduce(
                        agg[:], za_ps[:].rearrange("p (o i) -> p o i", o=32, i=32),
                        axis=mybir.AxisListType.X, op=ALU.add)
                    m_sb = sbp.tile([128, 32], F32, tag="m_sb")
                    nc.scalar.activation(m_sb[:], agg[:], AF.Relu,
                                         scale=invd_sb[:, w:w + 1])
                    mT_ps = psC.tile([32, 128], F32, tag="misc")
                    nc.tensor.transpose(mT_ps[:], m_sb[:], eye_sb[:, :])
                    nc.scalar.copy(mT_aug[0:32, w * 128:(w + 1) * 128], mT_ps[:])

                # GRU (feature-major); gi+gh summed via PSUM accumulation
                for f in range(NPAD // 512 if (dbg == 0 or dbg >= 5) else 0):
                    cols = slice(512 * f, 512 * (f + 1))
                    r_ps = psC.tile([32, 512], F32, tag="misc")
                    nc.tensor.matmul(r_ps[:], wih_sb[:, 0:32], mT_aug[:, cols],
                                     start=True, stop=False)
                    nc.tensor.matmul(r_ps[:], whh_sb[:, 0:32], hT_cur[:, cols],
                                     start=False, stop=True)
                    r_sb = grup.tile([32, 512], F32, tag="r_sb")
                    nc.scalar.activation(r_sb[:], r_ps[:], AF.Sigmoid)
                    ghn_ps = psC.tile([32, 512], F32, tag="misc")
                    nc.tensor.matmul(ghn_ps[:], whh_sb[:, 64:96], hT_cur[:, cols])
                    t1 = grup.tile([32, 512], F32, tag="t1")
                    nc.vector.tensor_mul(t1[:], r_sb[:], ghn_ps[:])
                    gin_ps = psC.tile([32, 512], F32, tag="misc")
                    nc.tensor.matmul(gin_ps[:], wih_sb[:, 64:96], mT_aug[:, cols])
                    t2 = grup.tile([32, 512], F32, tag="t2")
                    nc.vector.tensor_add(t2[:], t1[:], gin_ps[:])
                    z_ps = psC.tile([32, 512], F32, tag="misc")
                    nc.tensor.matmul(z_ps[:], wih_sb[:, 32:64], mT_aug[:, cols],
                                     start=True, stop=False)
                    nc.tensor.matmul(z_ps[:], whh_sb[:, 32:64], hT_cur[:, cols],
                                     start=False, stop=True)
                    z_sb = grup.tile([32, 512], F32, tag="z_sb")
                    nc.scalar.activation(z_sb[:], z_ps[:], AF.Sigmoid)
                    n_t = grup.tile([32, 512], F32, tag="n_t")
                    nc.scalar.activation(n_t[:], t2[:], AF.Tanh)
                    t3 = grup.tile([32, 512], F32, tag="t3")
                    nc.vector.tensor_sub(t3[:], hT_cur[0:32, cols], n_t[:])
                    t4 = grup.tile([32, 512], F32, tag="t4")
                    nc.vector.tensor_mul(t4[:], z_sb[:], t3[:])
                    nc.vector.tensor_add(hT_new[0:32, cols], n_t[:], t4[:])

                # transpose back to node-major; feed table (layer 0) / mean (layer 1)
                if layer == 1:
                    gs_ps = psB.tile([128, 32], F32, tag="za")
                for w in range(WIN if (dbg == 0 or dbg >= 5) else 0):
                    hn_ps = psC.tile([128, 32], F32, tag="misc")
                    nc.tensor.transpose(
                        hn_ps[:], hT_new[0:32, w * 128:(w + 1) * 128],
                        eye_sb[0:32, 0:32])
                    hn_sb = sbp.tile([128, 32], F32, tag="hn_sb")
                    nc.scalar.copy(hn_sb[:], hn_ps[:])
                    if layer == 0:
                        nc.sync.dma_start(
                            htab_sh1[w * 128:(w + 1) * 128, 0:32], hn_sb[:])
                    else:
                        m1 = sbp.tile([128, 128], F32, tag="m1")
                        nc.vector.tensor_scalar(
                            m1[:], iota_sb[:], gid_sb[:, w:w + 1], None,
                            ALU.is_equal)
                        nc.tensor.matmul(gs_ps[:], m1[:], hn_sb[:],
                                         start=(w == 0), stop=(w == WIN - 1))
                if layer == 0:
                    nc.gpsimd.collective_compute(
                        "AllGather", ALU.bypass, replica_groups=RG,
                        ins=[htab_sh1.opt()], outs=[htab1.opt()])
                hT_cur, hT_new = hT_new, hT_cur

            if dbg == 0:
                gs_sb = sbp.tile([128, 32], F32, tag="gs_sb")
                nc.scalar.copy(gs_sb[:], gs_ps[:])
                nc.sync.dma_start(out_d[:], gs_sb[:])
            else:
                gs_sb = sbp.tile([128, 32], F32, tag="gs_sb")
                nc.vector.memset(gs_sb[:], 0.0)
                nc.sync.dma_start(out_d[:], gs_sb[:])

    nc.compile()
    return nc


def _prep_inputs(n_feat, e_feat, src, dst, graph_ids,
                 W_atom, b_atom, W_bond, b_bond, W_e1, b_e1, W_e2, b_e2,
                 W_ih, W_hh, b_ih, b_hh):
    """Host-side sharding/index prep. Returns (t_w, in_maps, gcnt)."""
    src = np.asarray(src, np.int64)
    dst = np.asarray(dst, np.int64)
    graph_ids = np.asarray(graph_ids, np.int64)
    n_feat = np.asarray(n_feat, np.float32)
    e_feat = np.asarray(e_feat, np.float32)

    deg = np.bincount(dst, minlength=N).astype(np.float32)
    invd_full = 1.0 / np.maximum(deg, 1.0)

    order = np.argsort(dst, kind="stable")
    dst_s, src_s = dst[order], src[order]
    ef_s = e_feat[order]

    bounds = np.searchsorted(dst_s, np.arange(0, N + 1, NPC))
    win_of = (dst_s - (dst_s // NPC) * NPC) // 128

    # fixed tiles-per-window across all cores
    max_cnt = 0
    per_core = []
    for c in range(NCORES):
        lo, hi = bounds[c], bounds[c + 1]
        w = win_of[lo:hi]
        cnt = np.bincount(w, minlength=WIN)
        per_core.append((lo, hi, w, cnt))
        max_cnt = max(max_cnt, int(cnt.max()))
    t_w = -(-max_cnt // 128)
    if t_w % 2:
        t_w += 1
    nt = WIN * t_w
    e_pad = nt * 128

    # shared weight tensors
    bf = ml_dtypes.bfloat16
    wa = np.vstack([W_atom, b_atom[None, :]]).astype(np.float32)          # [65,32]
    wb = np.asarray(W_bond, np.float32)                                   # [16,32]
    bb = np.asarray(b_bond, np.float32)[:, None]                          # [32,1]
    we1 = np.vstack([W_e1, b_e1[None, :]]).astype(np.float32)             # [33,32]
    t_perm = np.asarray(W_e2, np.float32).reshape(32, 32, 32)             # [k,i,o]
    t_perm = np.transpose(t_perm, (0, 2, 1)).reshape(32, 1024)            # [k,(o,i)]
    b_perm = np.asarray(b_e2, np.float32).reshape(32, 32).T.reshape(1024)
    we2 = np.vstack([t_perm, b_perm[None, :]]).astype(bf)                 # [33,1024]
    wih = np.vstack([np.asarray(W_ih, np.float32).T, b_ih[None, :]])      # [33,96]
    whh = np.vstack([np.asarray(W_hh, np.float32).T, b_hh[None, :]])      # [33,96]
    iota = np.tile(np.arange(128, dtype=np.float32), (128, 1))
    eye = np.eye(128, dtype=np.float32)

    in_maps = []
    for c in range(NCORES):
        lo, hi, w, cnt = per_core[c]
        srcc, ef_c = src_s[lo:hi], ef_s[lo:hi]
        dloc = dst_s[lo:hi] - c * NPC - w * 128

        # positions in the padded per-core edge list (window-major)
        woff = np.concatenate([[0], np.cumsum(cnt)])[:-1]
        pos = w * (t_w * 128) + (np.arange(hi - lo) - woff[w])

        src_r = np.zeros(e_pad, np.int64)
        src_r[pos] = (srcc // NPC) * NPAD + srcc % NPC
        dstl = np.full(e_pad, -1.0, np.float32)
        dstl[pos] = dloc.astype(np.float32)
        ef_p = np.zeros((e_pad, 16), np.float32)
        ef_p[pos] = ef_c

        srcidx = np.tile(
            src_r.astype(np.int16).reshape(e_pad // 16, 16).T, (8, 1))

        vg = c * NPC + np.arange(NPAD)
        valid = vg < (c + 1) * NPC
        invd = np.where(valid, invd_full[np.minimum(vg, N - 1)], 0.0)
        gid = np.where(valid, graph_ids[np.minimum(vg, N - 1)], -1.0)

        nf_sh = np.zeros((NPAD, 64), np.float32)
        nf_sh[:NPC] = n_feat[c * NPC:(c + 1) * NPC]

        in_maps.append({
            "nfeat": nf_sh,
            "efeat": ef_p,
            "srcidx": srcidx,
            "dstl": dstl.reshape(nt, 128).T.copy(),
            "invd": invd.reshape(WIN, 128).T.astype(np.float32).copy(),
            "gid": gid.reshape(WIN, 128).T.astype(np.float32).copy(),
            "iota": iota, "eye": eye,
            "wa": wa, "wb": wb, "bb": bb, "we1": we1, "we2": we2,
            "wih": wih.astype(np.float32), "whh": whh.astype(np.float32),
        })

    gcnt = np.bincount(graph_ids, minlength=B).astype(np.float32)
    return t_w, in_maps, gcnt


def kernel(**inputs):
    t_w, in_maps, gcnt = _prep_inputs(**inputs)
    if t_w not in _graph_cache:
        _graph_cache[t_w] = _build_graph(t_w)
    nc = _graph_cache[t_w]
    res = run_bass_kernel_spmd(nc, in_maps, core_ids=list(range(NCORES)))
    gsum = np.zeros((B, H), np.float64)
    for r in res.results:
        gsum += r["out"].astype(np.float64)
    out = gsum / np.maximum(gcnt, 1.0)[:, None]
    return out.astype(np.float32)


if __name__ == "__main__":
    import reference
    inputs = {k: np.asarray(v) for k, v in reference.setup_inputs().items()}
    got = kernel(**inputs)
    exp = np.asarray(reference.reference(**inputs))
    rel = np.abs(got - exp).max() / np.abs(exp).max()
    print("Relative error:", rel)


# revision 9
# speedup vs baseline: 1.0625x; 1.0625x over previous
"""Distributed Trainium2 Bass kernel for nn_AtomEncoder (NNConv/GRU message passing).

Strategy (8 cores, SPMD):
  - Edges are dst-sharded: core c owns all edges whose dst is in [2500c, 2500(c+1)),
    sorted by dst, grouped into 20 windows of 128 nodes, padded to a fixed
    tiles-per-window so every core runs an identical instruction stream.
  - Node state h lives as a full replicated HBM table [8*2560, 64] (row padded to
    256B for dma_gather); each layer's updated shard is AllGathered.
  - Per edge tile (128 edges):
      ew  = r_aug @ W_e2aug            (PE, bf16; r = relu(edge-MLP), recomputed-free)
      z2  = ew * h[src] (broadcast)    (DVE)
      ZA += S^T @ z2                   (PE; S = one-hot(dst_local) built by is_equal)
    The i-reduction (msg = sum_i h_i * ew[i,o]) is deferred past the scatter:
    once per 128-node window, agg[v,o] = sum_i ZA[v,(o,i)]  (DVE strided reduce).
  - GRU runs feature-major (h^T [32, 2560]) so biases are per-partition ACT ops.
  - Graph mean via one-hot matmul accumulated in PSUM; per-core partials are
    summed and divided on the host (unshard step).
"""

import os
import sys

import numpy as np
import ml_dtypes

for _p in ("/opt/trn_rl_repo", "/root/.axon_site/_ro/trn_rl_repo"):
    if os.path.isdir(_p) and _p not in sys.path:
        sys.path.insert(0, _p)

import concourse.bacc as bacc  # noqa: E402
import concourse.bass as bass  # noqa: E402
import concourse.mybir as mybir  # noqa: E402
import concourse.tile as tile  # noqa: E402
from concourse.bass_utils import run_bass_kernel_spmd  # noqa: E402

NCORES = 8
N, E, B, H = 20000, 320000, 128, 32
NPC = N // NCORES          # 2500 nodes per core
WIN = 20                   # 128-node windows per core
NPAD = WIN * 128           # 2560 padded nodes per core
NTAB = NCORES * NPAD       # 20480 rows in the replicated h table

F32 = mybir.dt.float32
BF16 = mybir.dt.bfloat16
I16 = mybir.dt.int16

_graph_cache = {}


def _build_graph(t_w, trace=False):
    """Build the SPMD Bass graph. t_w = tiles per 128-node window (even)."""
    dbg = int(os.environ.get("KDBG", "0") or 0)
    nt = WIN * t_w               # edge tiles per core
    e_pad = nt * 128             # padded edges per core
    nb = e_pad // 1024           # 8-tile transpose batches for e_feat
    ch_tiles = 4                 # gather chunk = 4 tiles (512 idxs; 2304 crashes SWDGE)
    n_ch = nt // 4               # gather chunks per layer
    g = ch_tiles * 128           # idxs per gather chunk

    nc = bacc.Bacc("TRN2", target_bir_lowering=False, debug=False,
                   num_devices=NCORES)

    def inp(name, shape, dt=F32):
        return nc.dram_tensor(name, list(shape), dt, kind="ExternalInput").ap()

    nfeat_d = inp("nfeat", (NPAD, 64))
    efeat_d = inp("efeat", (e_pad, 16))
    srcidx_d = inp("srcidx", (128, e_pad // 16), I16)
    dstl_d = inp("dstl", (128, nt))
    invd_d = inp("invd", (128, WIN))
    gid_d = inp("gid", (128, WIN))
    iota_d = inp("iota", (128, 128))
    eye_d = inp("eye", (128, 128))
    wa_d = inp("wa", (65, 32))
    wb_d = inp("wb", (16, 32))
    bb_d = inp("bb", (32, 1))
    we1_d = inp("we1", (33, 32))
    we2_d = inp("we2", (33, 1024), BF16)
    wih_d = inp("wih", (33, 96))
    whh_d = inp("whh", (33, 96))
    out_d = nc.dram_tensor("out", [128, 32], F32, kind="ExternalOutput").ap()

    AF = mybir.ActivationFunctionType
    ALU = mybir.AluOpType
    RG = [list(range(NCORES))]

    with tile.TileContext(nc) as tc:
        with tc.tile_pool(name="constp", bufs=1) as constp, \
             tc.tile_pool(name="bigp", bufs=1) as bigp, \
             tc.tile_pool(name="sbp", bufs=2) as sbp, \
             tc.tile_pool(name="sbp3", bufs=3) as sbp3, \
             tc.tile_pool(name="grup", bufs=1) as grup, \
             tc.tile_pool(name="dramp", bufs=1, space="DRAM") as dramp, \
             tc.tile_pool(name="psA", bufs=2, space="PSUM") as psA, \
             tc.tile_pool(name="psB", bufs=1, space="PSUM") as psB, \
             tc.tile_pool(name="psC", bufs=2, space="PSUM") as psC:

            # ---- constants to SBUF ----
            def load_const(name, ap, shape, dt=F32):
                t = constp.tile(list(shape), dt, name=name)
                nc.sync.dma_start(t[:], ap)
                return t

            iota_sb = load_const("iota_sb", iota_d, (128, 128))
            eye_sb = load_const("eye_sb", eye_d, (128, 128))
            wa_sb = load_const("wa_sb", wa_d, (65, 32))
            wb_sb = load_const("wb_sb", wb_d, (16, 32))
            bb_sb = load_const("bb_sb", bb_d, (32, 1))
            we1_sb = load_const("we1_sb", we1_d, (33, 32))
            we2_sb = load_const("we2_sb", we2_d, (33, 1024), BF16)
            wih_sb = load_const("wih_sb", wih_d, (33, 96))
            whh_sb = load_const("whh_sb", whh_d, (33, 96))
            invd_sb = load_const("invd_sb", invd_d, (128, WIN))
            gid_sb = load_const("gid_sb", gid_d, (128, WIN))
            dstl_sb = load_const("dstl_sb", dstl_d, (128, nt))
            srcidx_sb = load_const("srcidx_sb", srcidx_d, (128, e_pad // 16), I16)

            # ---- persistent buffers ----
            r_augT = bigp.tile([33, e_pad], BF16, name="r_augT")
            hT_a = bigp.tile([33, NPAD], F32, name="hT_a")
            hT_b = bigp.tile([33, NPAD], F32, name="hT_b")
            mT_aug = bigp.tile([33, NPAD], F32, name="mT_aug")
            nc.vector.memset(r_augT[32:33, :], 1.0)
            nc.vector.memset(hT_a[32:33, :], 1.0)
            nc.vector.memset(hT_b[32:33, :], 1.0)
            nc.vector.memset(mT_aug[32:33, :], 1.0)

            htab_sh0 = dramp.tile([NPAD, 64], F32, name="htab_sh0")
            htab_sh1 = dramp.tile([NPAD, 64], F32, name="htab_sh1")
            htab0 = dramp.tile([NTAB, 64], F32, name="htab0", addr_space="Shared")
            htab1 = dramp.tile([NTAB, 64], F32, name="htab1", addr_space="Shared")

            # ---- P1: initial node embedding h0 ----
            for w in range(WIN):
                nf_t = sbp.tile([128, 64], F32, tag="nf_t")
                nc.sync.dma_start(nf_t[:], nfeat_d[w * 128:(w + 1) * 128, :])
                tr_ps = psC.tile([64, 128], F32, tag="misc")
                nc.tensor.transpose(tr_ps[:], nf_t[:], eye_sb[:, :])
                nfT = sbp.tile([65, 128], F32, tag="nfT")
                nc.scalar.copy(nfT[0:64, :], tr_ps[:])
                nc.vector.memset(nfT[64:65, :], 1.0)
                # feature-major h0^T into hT_a
                h0T_ps = psC.tile([32, 128], F32, tag="misc")
                nc.tensor.matmul(h0T_ps[:], wa_sb[:], nfT[:])
                nc.scalar.copy(hT_a[0:32, w * 128:(w + 1) * 128], h0T_ps[:])
                # node-major h0 into the gather table shard
                h0_ps = psC.tile([128, 32], F32, tag="misc")
                nc.tensor.matmul(h0_ps[:], nfT[:], wa_sb[:])
                h0_sb = sbp.tile([128, 32], F32, tag="hn_sb")
                nc.scalar.copy(h0_sb[:], h0_ps[:])
                nc.sync.dma_start(htab_sh0[w * 128:(w + 1) * 128, 0:32], h0_sb[:])

            nc.gpsimd.collective_compute(
                "AllGather", ALU.bypass, replica_groups=RG,
                ins=[htab_sh0.opt()], outs=[htab0.opt()])

            # ---- P2: edge MLP r = relu(h_bond @ W_e1 + b_e1), feature-major ----
            for t in range(nt if (dbg == 0 or dbg >= 2) else 0):
                col = t * 128
                ef_t = sbp3.tile([128, 16], F32, tag="ef_t")
                nc.sync.dma_start(ef_t[:], efeat_d[col:col + 128, :])
                tr2_ps = psC.tile([16, 128], F32, tag="misc")
                nc.tensor.transpose(tr2_ps[:], ef_t[:], eye_sb[:, :])
                efT = sbp3.tile([16, 128], F32, tag="efT")
                nc.scalar.copy(efT[:], tr2_ps[:])
                hb_ps = psC.tile([32, 128], F32, tag="misc")
                nc.tensor.matmul(hb_ps[:], wb_sb[:], efT[:])
                hba = sbp3.tile([33, 128], F32, tag="hba")
                nc.vector.tensor_scalar_add(hba[0:32, :], hb_ps[:], bb_sb[:])
                nc.vector.memset(hba[32:33, :], 1.0)
                r_ps = psC.tile([32, 128], F32, tag="misc")
                nc.tensor.matmul(r_ps[:], we1_sb[:], hba[:])
                nc.scalar.activation(r_augT[0:32, col:col + 128], r_ps[:], AF.Relu)

            # ---- P3: message-passing layers ----
            hT_cur, hT_new = hT_a, hT_b
            htabs = [htab0, htab1]
            htab_shs = [htab_sh0, htab_sh1]
            n_layers = 2 if (dbg == 0 or dbg >= 5) else (1 if dbg >= 3 else 0)
            for layer in range(n_layers):
                htab = htabs[layer]
                nc.vector.memset(mT_aug[32:33, :], 1.0)
                hs_chunks = []
                for ch in range(n_ch):
                    hs = sbp.tile([128, ch_tiles, 64], F32, tag="hs")
                    nc.gpsimd.dma_gather(
                        hs[:], htab[:],
                        srcidx_sb[:, ch * (g // 16):(ch + 1) * (g // 16)],
                        g, g, 64)
                    hs_chunks.append(hs)

                for w in range(WIN if (dbg == 0 or dbg >= 4) else 0):
                    za_ps = psB.tile([128, 1024], F32, tag="za")
                    for i in range(t_w):
                        t = w * t_w + i
                        hs = hs_chunks[t // ch_tiles]
                        tt = t % ch_tiles
                        ew_ps = psA.tile([128, 1024], F32, tag="ew")
                        lhs_r = r_augT[0:33, t * 128:(t + 1) * 128]
                        nc.tensor.matmul(ew_ps[:, 0:512], lhs_r, we2_sb[:, 0:512])
                        nc.tensor.matmul(ew_ps[:, 512:1024], lhs_r, we2_sb[:, 512:1024])
                        hb = hs[:, tt, 0:32].unsqueeze(1).broadcast_to([128, 32, 32])
                        if i % 3 == 2:
                            # Pool path: ACT stages ew out of PSUM, GPSIMD multiplies
                            ew_sb = sbp.tile([128, 1024], F32, tag="ew_sb")
                            nc.scalar.copy(ew_sb[:], ew_ps[:])
                            z2f = sbp.tile([128, 1024], F32, tag="z2f")
                            nc.gpsimd.tensor_tensor(
                                z2f[:].rearrange("p (o i) -> p o i", o=32, i=32),
                                ew_sb[:].rearrange("p (o i) -> p o i", o=32, i=32),
                                hb, ALU.mult)
                            s32 = sbp.tile([128, 128], F32, tag="s32")
                            nc.vector.tensor_scalar(
                                s32[:], iota_sb[:], dstl_sb[:, t:t + 1], None,
                                ALU.is_equal)
                            nc.tensor.matmul(za_ps[:, 0:512], s32[:], z2f[:, 0:512],
                                             start=(i == 0), stop=(i == t_w - 1))
                            nc.tensor.matmul(za_ps[:, 512:1024], s32[:], z2f[:, 512:1024],
                                             start=(i == 0), stop=(i == t_w - 1))
                        else:
                            z2 = sbp3.tile([128, 1024], BF16, tag="z2")
                            nc.vector.tensor_tensor(
                                z2[:].rearrange("p (o i) -> p o i", o=32, i=32),
                                ew_ps[:].rearrange("p (o i) -> p o i", o=32, i=32),
                                hb, ALU.mult)
                            s_oh = sbp3.tile([128, 128], BF16, tag="s_oh")
                            nc.vector.tensor_scalar(
                                s_oh[:], iota_sb[:], dstl_sb[:, t:t + 1], None,
                                ALU.is_equal)
                            nc.tensor.matmul(za_ps[:, 0:512], s_oh[:], z2[:, 0:512],
                                             start=(i == 0), stop=(i == t_w - 1))
                            nc.tensor.matmul(za_ps[:, 512:1024], s_oh[:], z2[:, 512:1024],
                                             start=(i == 0), stop=(i == t_w - 1))
                    # window epilogue: reduce over i, scale by 1/deg, relu, transpose
                    agg = sbp.tile([128, 32], F32, tag="agg")
                    nc.vector.tensor_reduce(
                        agg[:], za_ps[:].rearrange("p (o i) -> p o i", o=32, i=32),
                        axis=mybir.AxisListType.X, op=ALU.add)
                    m_sb = sbp.tile([128, 32], F32, tag="m_sb")
                    nc.scalar.activation(m_sb[:], agg[:], AF.Relu,
                                         scale=invd_sb[:, w:w + 1])
                    mT_ps = psC.tile([32, 128], F32, tag="misc")
                    nc.tensor.transpose(mT_ps[:], m_sb[:], eye_sb[:, :])
                    nc.scalar.copy(mT_aug[0:32, w * 128:(w + 1) * 128], mT_ps[:])

                # GRU (feature-major); gi+gh summed via PSUM accumulation
                for f in range(NPAD // 512 if (dbg == 0 or dbg >= 5) else 0):
                    cols = slice(512 * f, 512 * (f + 1))
                    r_ps = psC.tile([32, 512], F32, tag="misc")
                    nc.tensor.matmul(r_ps[:], wih_sb[:, 0:32], mT_aug[:, cols],
                                     start=True, stop=False)
                    nc.tensor.matmul(r_ps[:], whh_sb[:, 0:32], hT_cur[:, cols],
                                     start=False, stop=True)
                    r_sb = grup.tile([32, 512], F32, tag="r_sb")
                    nc.scalar.activation(r_sb[:], r_ps[:], AF.Sigmoid)
                    ghn_ps = psC.tile([32, 512], F32, tag="misc")
                    nc.tensor.matmul(ghn_ps[:], whh_sb[:, 64:96], hT_cur[:, cols])
                    t1 = grup.tile([32, 512], F32, tag="t1")
                    nc.vector.tensor_mul(t1[:], r_sb[:], ghn_ps[:])
                    gin_ps = psC.tile([32, 512], F32, tag="misc")
                    nc.tensor.matmul(gin_ps[:], wih_sb[:, 64:96], mT_aug[:, cols])
                    t2 = grup.tile([32, 512], F32, tag="t2")
                    nc.vector.tensor_add(t2[:], t1[:], gin_ps[:])
                    z_ps = psC.tile([32, 512], F32, tag="misc")
                    nc.tensor.matmul(z_ps[:], wih_sb[:, 32:64], mT_aug[:, cols],
                                     start=True, stop=False)
                    nc.tensor.matmul(z_ps[:], whh_sb[:, 32:64], hT_cur[:, cols],
                                     start=False, stop=True)
                    z_sb = grup.tile([32, 512], F32, tag="z_sb")
                    nc.scalar.activation(z_sb[:], z_ps[:], AF.Sigmoid)
                    n_t = grup.tile([32, 512], F32, tag="n_t")
                    nc.scalar.activation(n_t[:], t2[:], AF.Tanh)
                    t3 = grup.tile([32, 512], F32, tag="t3")
                    nc.vector.tensor_sub(t3[:], hT_cur[0:32, cols], n_t[:])
                    t4 = grup.tile([32, 512], F32, tag="t4")
                    nc.vector.tensor_mul(t4[:], z_sb[:], t3[:])
                    nc.vector.tensor_add(hT_new[0:32, cols], n_t[:], t4[:])

                # transpose back to node-major; feed table (layer 0) / mean (layer 1)
                if layer == 1:
                    gs_ps = psB.tile([128, 32], F32, tag="za")
                for w in range(WIN if (dbg == 0 or dbg >= 5) else 0):
                    hn_ps = psC.tile([128, 32], F32, tag="misc")
                    nc.tensor.transpose(
                        hn_ps[:], hT_new[0:32, w * 128:(w + 1) * 128],
                        eye_sb[0:32, 0:32])
                    hn_sb = sbp.tile([128, 32], F32, tag="hn_sb")
                    nc.scalar.copy(hn_sb[:], hn_ps[:])
                    if layer == 0:
                        nc.sync.dma_start(
                            htab_sh1[w * 128:(w + 1) * 128, 0:32], hn_sb[:])
                    else:
                        m1 = sbp.tile([128, 128], F32, tag="m1")
                        nc.vector.tensor_scalar(
                            m1[:], iota_sb[:], gid_sb[:, w:w + 1], None,
                            ALU.is_equal)
                        nc.tensor.matmul(gs_ps[:], m1[:], hn_sb[:],
                                         start=(w == 0), stop=(w == WIN - 1))
                if layer == 0:
                    nc.gpsimd.collective_compute(
                        "AllGather", ALU.bypass, replica_groups=RG,
                        ins=[htab_sh1.opt()], outs=[htab1.opt()])
                hT_cur, hT_new = hT_new, hT_cur

            if dbg == 0:
                gs_sb = sbp.tile([128, 32], F32, tag="gs_sb")
                nc.scalar.copy(gs_sb[:], gs_ps[:])
                nc.sync.dma_start(out_d[:], gs_sb[:])
            else:
                gs_sb = sbp.tile([128, 32], F32, tag="gs_sb")
                nc.vector.memset(gs_sb[:], 0.0)
                nc.sync.dma_start(out_d[:], gs_sb[:])

    nc.compile()
    return nc


def _prep_inputs(n_feat, e_feat, src, dst, graph_ids,
                 W_atom, b_atom, W_bond, b_bond, W_e1, b_e1, W_e2, b_e2,
                 W_ih, W_hh, b_ih, b_hh):
    """Host-side sharding/index prep. Returns (t_w, in_maps, gcnt)."""
    src = np.asarray(src, np.int64)
    dst = np.asarray(dst, np.int64)
    graph_ids = np.asarray(graph_ids, np.int64)
    n_feat = np.asarray(n_feat, np.float32)
    e_feat = np.asarray(e_feat, np.float32)

    deg = np.bincount(dst, minlength=N).astype(np.float32)
    invd_full = 1.0 / np.maximum(deg, 1.0)

    order = np.argsort(dst, kind="stable")
    dst_s, src_s = dst[order], src[order]
    ef_s = e_feat[order]

    bounds = np.searchsorted(dst_s, np.arange(0, N + 1, NPC))
    win_of = (dst_s - (dst_s // NPC) * NPC) // 128

    # fixed tiles-per-window across all cores
    max_cnt = 0
    per_core = []
    for c in range(NCORES):
        lo, hi = bounds[c], bounds[c + 1]
        w = win_of[lo:hi]
        cnt = np.bincount(w, minlength=WIN)
        per_core.append((lo, hi, w, cnt))
        max_cnt = max(max_cnt, int(cnt.max()))
    t_w = -(-max_cnt // 128)
    if t_w % 2:
        t_w += 1
    nt = WIN * t_w
    e_pad = nt * 128

    # shared weight tensors
    bf = ml_dtypes.bfloat16
    wa = np.vstack([W_atom, b_atom[None, :]]).astype(np.float32)          # [65,32]
    wb = np.asarray(W_bond, np.float32)                                   # [16,32]
    bb = np.asarray(b_bond, np.float32)[:, None]                          # [32,1]
    we1 = np.vstack([W_e1, b_e1[None, :]]).astype(np.float32)             # [33,32]
    t_perm = np.asarray(W_e2, np.float32).reshape(32, 32, 32)             # [k,i,o]
    t_perm = np.transpose(t_perm, (0, 2, 1)).reshape(32, 1024)            # [k,(o,i)]
    b_perm = np.asarray(b_e2, np.float32).reshape(32, 32).T.reshape(1024)
    we2 = np.vstack([t_perm, b_perm[None, :]]).astype(bf)                 # [33,1024]
    wih = np.vstack([np.asarray(W_ih, np.float32).T, b_ih[None, :]])      # [33,96]
    whh = np.vstack([np.asarray(W_hh, np.float32).T, b_hh[None, :]])      # [33,96]
    iota = np.tile(np.arange(128, dtype=np.float32), (128, 1))
    eye = np.eye(128, dtype=np.float32)

    in_maps = []
    for c in range(NCORES):
        lo, hi, w, cnt = per_core[c]
        srcc, ef_c = src_s[lo:hi], ef_s[lo:hi]
        dloc = dst_s[lo:hi] - c * NPC - w * 128

        # positions in the padded per-core edge list (window-major)
        woff = np.concatenate([[0], np.cumsum(cnt)])[:-1]
        pos = w * (t_w * 128) + (np.arange(hi - lo) - woff[w])

        src_r = np.zeros(e_pad, np.int64)
        src_r[pos] = (srcc // NPC) * NPAD + srcc % NPC
        dstl = np.full(e_pad, -1.0, np.float32)
        dstl[pos] = dloc.astype(np.float32)
        ef_p = np.zeros((e_pad, 16), np.float32)
        ef_p[pos] = ef_c

        srcidx = np.tile(
            src_r.astype(np.int16).reshape(e_pad // 16, 16).T, (8, 1))

        vg = c * NPC + np.arange(NPAD)
        valid = vg < (c + 1) * NPC
        invd = np.where(valid, invd_full[np.minimum(vg, N - 1)], 0.0)
        gid = np.where(valid, graph_ids[np.minimum(vg, N - 1)], -1.0)

        nf_sh = np.zeros((NPAD, 64), np.float32)
        nf_sh[:NPC] = n_feat[c * NPC:(c + 1) * NPC]

        in_maps.append({
            "nfeat": nf_sh,
            "efeat": ef_p,
            "srcidx": srcidx,
            "dstl": dstl.reshape(nt, 128).T.copy(),
            "invd": invd.reshape(WIN, 128).T.astype(np.float32).copy(),
            "gid": gid.reshape(WIN, 128).T.astype(np.float32).copy(),
            "iota": iota, "eye": eye,
            "wa": wa, "wb": wb, "bb": bb, "we1": we1, "we2": we2,
            "wih": wih.astype(np.float32), "whh": whh.astype(np.float32),
        })

    gcnt = np.bincount(graph_ids, minlength=B).astype(np.float32)
    return t_w, in_maps, gcnt


def kernel(**inputs):
    t_w, in_maps, gcnt = _prep_inputs(**inputs)
    if t_w not in _graph_cache:
        _graph_cache[t_w] = _build_graph(t_w)
    nc = _graph_cache[t_w]
    res = run_bass_kernel_spmd(nc, in_maps, core_ids=list(range(NCORES)))
    gsum = np.zeros((B, H), np.float64)
    for r in res.results:
        gsum += r["out"].astype(np.float64)
    out = gsum / np.maximum(gcnt, 1.0)[:, None]
    return out.astype(np.float32)


if __name__ == "__main__":
    import reference
    inputs = {k: np.asarray(v) for k, v in reference.setup_inputs().items()}
    got = kernel(**inputs)
    exp = np.asarray(reference.reference(**inputs))
    rel = np.abs(got - exp).max() / np.abs(exp).max()
    print("Relative error:", rel)
